# revision 4
# baseline (speedup 1.0000x reference)
"""Single-head attention (B=8, S=2048, D=512) on 8 TRN2 NeuronCores.

Sharding: data-parallel over batch — core i computes batch element i
entirely locally (no collectives). Host-side prep is layout only
(transpose/reshape of the f32 shards); all compute (casts, projections,
attention, softmax, output projection) runs on-device.

Math (per core, x = x[b] of shape [S, D]):
  Q^T[e,s] = sum_d WqT[d,e] xT[d,s] + bq[e]      (bf16 matmul, f32 psum)
  K^T, V analogous; V kept as [s,e].
  S^T[k,q] = sum_e K^T[e,k] Q^T[e,q]             (scores, transposed layout)
  E = exp(S^T / sqrt(D))                          (ScalarE, no max-sub: scores
                                                   are O(30) max -> exp fits f32)
  denom[q] = sum_k E[k,q] via all-ones matmul -> replicated over partitions
  ctx^T[e,q] = sum_k V[k,e] E[k,q]; normalized by 1/denom during psum->sbuf
  out[s,o] = sum_e ctx^T[e,s] WoT[e,o] + bo      -> DMA to DRAM
"""

import sys

if "/opt/trn_rl_repo" not in sys.path:
    sys.path.insert(0, "/opt/trn_rl_repo")

import math

import numpy as np

import concourse.bass as bass
import concourse.mybir as mybir
import concourse.tile as tile

from concourse import bacc
from concourse.tile import TileContext

N_CORES = 8
S = 2048
D = 512
DO = 512

P = 128          # partition tile
F = 512          # free-dim tile (psum bank = 512 f32)
DC = D // P      # 4 contraction chunks over d
EC = D // P      # 4 chunks over e
SC = S // P      # 16 chunks over s (=k)
QB = S // F      # 4 q blocks of 512
KC = S // P      # 16 k chunks

F32 = mybir.dt.float32
BF16 = mybir.dt.bfloat16

_SCALE = 1.0 / math.sqrt(D)


def build():
    nc = bacc.Bacc(None)

    xT_e = nc.dram_tensor("xT", [D, S], F32, kind="ExternalInput")
    WqT_e = nc.dram_tensor("WqT", [D, D], F32, kind="ExternalInput")
    WkT_e = nc.dram_tensor("WkT", [D, D], F32, kind="ExternalInput")
    WvT_e = nc.dram_tensor("WvT", [D, D], F32, kind="ExternalInput")
    WoT_e = nc.dram_tensor("WoT", [D, DO], F32, kind="ExternalInput")
    bq_e = nc.dram_tensor("bq_col", [D, 1], F32, kind="ExternalInput")
    bk_e = nc.dram_tensor("bk_col", [D, 1], F32, kind="ExternalInput")
    bv_e = nc.dram_tensor("bv_row", [1, D], F32, kind="ExternalInput")
    bo_e = nc.dram_tensor("bo_row", [1, DO], F32, kind="ExternalInput")
    out_e = nc.dram_tensor("out", [S, DO], F32, kind="ExternalOutput")

    with TileContext(nc) as tc:
        with (
            tc.tile_pool(name="io", bufs=2) as io,
            tc.tile_pool(name="persist", bufs=1) as ps,
            tc.tile_pool(name="expp", bufs=2) as expp,
            tc.tile_pool(name="recipp", bufs=2) as recipp,
            tc.tile_pool(name="outp", bufs=3) as outp,
            tc.tile_pool(name="psA", bufs=2, space="PSUM") as psA,
            tc.tile_pool(name="psS", bufs=2, space="PSUM") as psS,
            tc.tile_pool(name="psD", bufs=2, space="PSUM") as psD,
            tc.tile_pool(name="psC", bufs=2, space="PSUM") as psC,
        ):
            # ---- load + cast inputs -------------------------------------
            xT = []
            for dc in range(DC):
                xl = io.tile([P, S], F32, tag="xload", name="xload")
                nc.sync.dma_start(xl[:], xT_e[dc * P : (dc + 1) * P, :])
                xb = ps.tile([P, S], BF16, tag=f"xT{dc}", name=f"xT{dc}")
                nc.vector.tensor_copy(xb[:], xl[:])
                xT.append(xb)

            def load_w(ext, name, ncols):
                tiles = []
                for dc in range(DC):
                    wl = io.tile([P, ncols], F32, tag="wload", name="wload")
                    nc.sync.dma_start(wl[:], ext[dc * P : (dc + 1) * P, :])
                    wb = ps.tile([P, ncols], BF16, tag=f"{name}{dc}", name=f"{name}{dc}")
                    nc.vector.tensor_copy(wb[:], wl[:])
                    tiles.append(wb)
                return tiles

            WqT = load_w(WqT_e, "WqT", D)
            WkT = load_w(WkT_e, "WkT", D)
            WvT = load_w(WvT_e, "WvT", D)
            WoT = load_w(WoT_e, "WoT", DO)

            bqc = ps.tile([P, DC], F32, tag="bqc", name="bqc")
            bkc = ps.tile([P, DC], F32, tag="bkc", name="bkc")
            for j in range(DC):
                nc.sync.dma_start(bqc[:, j : j + 1], bq_e[j * P : (j + 1) * P, :])
                nc.sync.dma_start(bkc[:, j : j + 1], bk_e[j * P : (j + 1) * P, :])

            bvl = io.tile([1, D], F32, tag="brow", name="brow")
            nc.sync.dma_start(bvl[:], bv_e[:, :])
            bvr = ps.tile([1, D], BF16, tag="bvr", name="bvr")
            nc.vector.tensor_copy(bvr[:], bvl[:])
            bol = io.tile([1, DO], F32, tag="brow", name="brow")
            nc.sync.dma_start(bol[:], bo_e[:, :])
            bor = ps.tile([1, DO], BF16, tag="bor", name="bor")
            nc.vector.tensor_copy(bor[:], bol[:])

            ones1 = ps.tile([1, P], BF16, tag="ones1", name="ones1")
            nc.any.memset(ones1[:], 1.0)
            ones128 = ps.tile([P, P], BF16, tag="ones128", name="ones128")
            nc.any.memset(ones128[:], 1.0)

            # ---- QKV projections ----------------------------------------
            QT = [ps.tile([P, S], BF16, tag=f"QT{ec}", name=f"QT{ec}") for ec in range(EC)]
            KT = [ps.tile([P, S], BF16, tag=f"KT{ec}", name=f"KT{ec}") for ec in range(EC)]
            V = [ps.tile([P, D], BF16, tag=f"V{sc}", name=f"V{sc}") for sc in range(SC)]
            ctxT = [ps.tile([P, S], BF16, tag=f"ctxT{ec}", name=f"ctxT{ec}") for ec in range(EC)]

            for ec in range(EC):
                es = slice(ec * P, (ec + 1) * P)
                for sb in range(QB):
                    ss = slice(sb * F, (sb + 1) * F)
                    pq = psA.tile([P, F], F32, tag="psA", name="psA")
                    for dc in range(DC):
                        nc.tensor.matmul(
                            pq[:], WqT[dc][:, es], xT[dc][:, ss],
                            start=(dc == 0), stop=(dc == DC - 1),
                        )
                    nc.scalar.add(QT[ec][:, ss], pq[:], bqc[:, ec : ec + 1])
                    pk = psA.tile([P, F], F32, tag="psA", name="psA")
                    for dc in range(DC):
                        nc.tensor.matmul(
                            pk[:], WkT[dc][:, es], xT[dc][:, ss],
                            start=(dc == 0), stop=(dc == DC - 1),
                        )
                    nc.scalar.add(KT[ec][:, ss], pk[:], bkc[:, ec : ec + 1])

            for sc in range(SC):
                scs = slice(sc * P, (sc + 1) * P)
                pv = psA.tile([P, D], F32, tag="psA", name="psA")
                for dc in range(DC):
                    nc.tensor.matmul(
                        pv[:], xT[dc][:, scs], WvT[dc][:],
                        start=(dc == 0), stop=False,
                    )
                nc.tensor.matmul(pv[:], ones1[:], bvr[:], start=False, stop=True)
                nc.vector.tensor_copy(V[sc][:], pv[:])

            # ---- attention, blocked over q ------------------------------
            for qb in range(QB):
                qs = slice(qb * F, (qb + 1) * F)
                eblk = expp.tile([P, KC * F], BF16, tag="expblk", name="expblk")
                for kc in range(KC):
                    ks = slice(kc * P, (kc + 1) * P)
                    pss = psS.tile([P, F], F32, tag="psS", name="psS")
                    for ec in range(EC):
                        nc.tensor.matmul(
                            pss[:], KT[ec][:, ks], QT[ec][:, qs],
                            start=(ec == 0), stop=(ec == EC - 1),
                        )
                    nc.scalar.activation(
                        eblk[:, kc * F : (kc + 1) * F], pss[:],
                        mybir.ActivationFunctionType.Exp, scale=_SCALE,
                    )

                pd = psD.tile([P, F], F32, tag="psD", name="psD")
                for kc in range(KC):
                    nc.tensor.matmul(
                        pd[:], ones128[:], eblk[:, kc * F : (kc + 1) * F],
                        start=(kc == 0), stop=(kc == KC - 1),
                    )
                recip = recipp.tile([P, F], F32, tag="recip", name="recip")
                nc.vector.reciprocal(recip[:], pd[:])

                for ec in range(EC):
                    es = slice(ec * P, (ec + 1) * P)
                    pc = psC.tile([P, F], F32, tag="psC", name="psC")
                    for kc in range(KC):
                        nc.tensor.matmul(
                            pc[:], V[kc][:, es], eblk[:, kc * F : (kc + 1) * F],
                            start=(kc == 0), stop=(kc == KC - 1),
                        )
                    nc.vector.tensor_mul(ctxT[ec][:, qs], pc[:], recip[:])

                for sj in range(QB):
                    s0 = qb * F + sj * P
                    po = psA.tile([P, DO], F32, tag="psA", name="psA")
                    for ec in range(EC):
                        nc.tensor.matmul(
                            po[:], ctxT[ec][:, s0 : s0 + P], WoT[ec][:],
                            start=(ec == 0), stop=False,
                        )
                    nc.tensor.matmul(po[:], ones1[:], bor[:], start=False, stop=True)
                    ot = outp.tile([P, DO], F32, tag="out", name="outtile")
                    nc.scalar.copy(ot[:], po[:])
                    nc.sync.dma_start(out_e[s0 : s0 + P, :], ot[:])

    nc.compile()
    return nc


_NC = None


def _get_nc():
    global _NC
    if _NC is None:
        _NC = build()
    return _NC


def _make_in_maps(x, Wq, bq, Wk, bk, Wv, bv, Wo, bo):
    # Layout-only host prep: per-core shard = one batch element, transposed
    # weight/activation layouts (f32 throughout; casts happen on device).
    WqT = np.ascontiguousarray(np.asarray(Wq, np.float32).T)
    WkT = np.ascontiguousarray(np.asarray(Wk, np.float32).T)
    WvT = np.ascontiguousarray(np.asarray(Wv, np.float32).T)
    WoT = np.ascontiguousarray(np.asarray(Wo, np.float32).T)
    bq_col = np.ascontiguousarray(np.asarray(bq, np.float32).reshape(D, 1))
    bk_col = np.ascontiguousarray(np.asarray(bk, np.float32).reshape(D, 1))
    bv_row = np.ascontiguousarray(np.asarray(bv, np.float32).reshape(1, D))
    bo_row = np.ascontiguousarray(np.asarray(bo, np.float32).reshape(1, DO))
    in_maps = []
    for i in range(N_CORES):
        in_maps.append(
            {
                "xT": np.ascontiguousarray(np.asarray(x[i], np.float32).T),
                "WqT": WqT,
                "WkT": WkT,
                "WvT": WvT,
                "WoT": WoT,
                "bq_col": bq_col,
                "bk_col": bk_col,
                "bv_row": bv_row,
                "bo_row": bo_row,
            }
        )
    return in_maps


def run(inputs, trace=False):
    """Compile (cached) + run on cores 0-7. Returns (output, BassKernelResults)."""
    from concourse.bass_utils import run_bass_kernel_spmd

    nc = _get_nc()
    in_maps = _make_in_maps(**inputs)
    res = run_bass_kernel_spmd(
        nc, in_maps, core_ids=list(range(N_CORES)), trace=trace
    )
    out = np.stack([res.results[i]["out"] for i in range(N_CORES)], axis=0)
    return out.astype(np.float32), res


def kernel(**inputs) -> np.ndarray:
    out, _ = run(inputs, trace=False)
    return out


# revision 6
# speedup vs baseline: 1.0691x; 1.0691x over previous
"""Single-head attention (B=8, S=2048, D=512) on 8 TRN2 NeuronCores.

Sharding: data-parallel over batch — core i computes batch element i
entirely locally (no collectives). Host-side prep is layout only
(transpose/reshape of the f32 shards); all compute (casts, projections,
attention, softmax, output projection) runs on-device.

Math (per core, x = x[b] of shape [S, D]):
  Q^T[e,s] = sum_d WqT[d,e] xT[d,s] + bq[e]      (bf16 matmul, f32 psum)
  K^T, V analogous; V kept as [s,e].
  S^T[k,q] = sum_e K^T[e,k] Q^T[e,q]             (scores, transposed layout)
  E = exp(S^T / sqrt(D))                          (ScalarE, no max-sub: scores
                                                   are O(30) max -> exp fits f32)
  denom[q] = sum_k E[k,q] via all-ones matmul -> replicated over partitions
  ctx^T[e,q] = sum_k V[k,e] E[k,q]; normalized by 1/denom during psum->sbuf
  out[s,o] = sum_e ctx^T[e,s] WoT[e,o] + bo      -> DMA to DRAM
"""

import sys

if "/opt/trn_rl_repo" not in sys.path:
    sys.path.insert(0, "/opt/trn_rl_repo")

import math

import numpy as np

import concourse.bass as bass
import concourse.mybir as mybir
import concourse.tile as tile

from concourse import bacc
from concourse.tile import TileContext

N_CORES = 8
S = 2048
D = 512
DO = 512

P = 128          # partition tile
F = 512          # free-dim tile (psum bank = 512 f32)
DC = D // P      # 4 contraction chunks over d
EC = D // P      # 4 chunks over e
SC = S // P      # 16 chunks over s (=k)
QB = S // F      # 4 q blocks of 512
KC = S // P      # 16 k chunks

F32 = mybir.dt.float32
BF16 = mybir.dt.bfloat16

_SCALE = 1.0 / math.sqrt(D)


def build():
    nc = bacc.Bacc(None)

    xT_e = nc.dram_tensor("xT", [D, S], F32, kind="ExternalInput")
    WqT_e = nc.dram_tensor("WqT", [D, D], F32, kind="ExternalInput")
    WkT_e = nc.dram_tensor("WkT", [D, D], F32, kind="ExternalInput")
    WvT_e = nc.dram_tensor("WvT", [D, D], F32, kind="ExternalInput")
    WoT_e = nc.dram_tensor("WoT", [D, DO], F32, kind="ExternalInput")
    bq_e = nc.dram_tensor("bq_col", [D, 1], F32, kind="ExternalInput")
    bk_e = nc.dram_tensor("bk_col", [D, 1], F32, kind="ExternalInput")
    bv_e = nc.dram_tensor("bv_row", [1, D], F32, kind="ExternalInput")
    bo_e = nc.dram_tensor("bo_row", [1, DO], F32, kind="ExternalInput")
    out_e = nc.dram_tensor("out", [S, DO], F32, kind="ExternalOutput")

    with TileContext(nc) as tc:
        with (
            tc.tile_pool(name="io", bufs=2) as io,
            tc.tile_pool(name="persist", bufs=1) as ps,
            tc.tile_pool(name="expp", bufs=2) as expp,
            tc.tile_pool(name="recipp", bufs=2) as recipp,
            tc.tile_pool(name="outp", bufs=3) as outp,
            tc.tile_pool(name="psA", bufs=2, space="PSUM") as psA,
            tc.tile_pool(name="psS", bufs=3, space="PSUM") as psS,
            tc.tile_pool(name="psC", bufs=3, space="PSUM") as psC,
        ):
            # ---- load + cast inputs -------------------------------------
            xT = []
            for dc in range(DC):
                xl = io.tile([P, S], F32, tag="xload", name="xload")
                nc.sync.dma_start(xl[:], xT_e[dc * P : (dc + 1) * P, :])
                xb = ps.tile([P, S], BF16, tag=f"xT{dc}", name=f"xT{dc}")
                nc.vector.tensor_copy(xb[:], xl[:])
                xT.append(xb)

            def load_w(ext, name, ncols):
                tiles = []
                for dc in range(DC):
                    wl = io.tile([P, ncols], F32, tag="wload", name="wload")
                    nc.sync.dma_start(wl[:], ext[dc * P : (dc + 1) * P, :])
                    wb = ps.tile([P, ncols], BF16, tag=f"{name}{dc}", name=f"{name}{dc}")
                    nc.vector.tensor_copy(wb[:], wl[:])
                    tiles.append(wb)
                return tiles

            WqT = load_w(WqT_e, "WqT", D)
            WkT = load_w(WkT_e, "WkT", D)
            WvT = load_w(WvT_e, "WvT", D)
            WoT = load_w(WoT_e, "WoT", DO)

            bqc = ps.tile([P, DC], F32, tag="bqc", name="bqc")
            bkc = ps.tile([P, DC], F32, tag="bkc", name="bkc")
            for j in range(DC):
                nc.sync.dma_start(bqc[:, j : j + 1], bq_e[j * P : (j + 1) * P, :])
                nc.sync.dma_start(bkc[:, j : j + 1], bk_e[j * P : (j + 1) * P, :])

            bvl = io.tile([1, D], F32, tag="brow", name="brow")
            nc.sync.dma_start(bvl[:], bv_e[:, :])
            bvr = ps.tile([1, D], BF16, tag="bvr", name="bvr")
            nc.vector.tensor_copy(bvr[:], bvl[:])
            bol = io.tile([1, DO], F32, tag="brow", name="brow")
            nc.sync.dma_start(bol[:], bo_e[:, :])
            bor = ps.tile([1, DO], BF16, tag="bor", name="bor")
            nc.vector.tensor_copy(bor[:], bol[:])

            ones1 = ps.tile([1, P], BF16, tag="ones1", name="ones1")
            nc.any.memset(ones1[:], 1.0)
            ones128 = ps.tile([P, P], BF16, tag="ones128", name="ones128")
            nc.any.memset(ones128[:], 1.0)

            # ---- QKV projections ----------------------------------------
            QT = [ps.tile([P, S], BF16, tag=f"QT{ec}", name=f"QT{ec}") for ec in range(EC)]
            KT = [ps.tile([P, S], BF16, tag=f"KT{ec}", name=f"KT{ec}") for ec in range(EC)]
            V = [ps.tile([P, D], BF16, tag=f"V{sc}", name=f"V{sc}") for sc in range(SC)]
            ctxT = [ps.tile([P, S], BF16, tag=f"ctxT{ec}", name=f"ctxT{ec}") for ec in range(EC)]

            for ec in range(EC):
                es = slice(ec * P, (ec + 1) * P)
                for sb in range(QB):
                    ss = slice(sb * F, (sb + 1) * F)
                    pq = psA.tile([P, F], F32, tag="psA", name="psA")
                    for dc in range(DC):
                        nc.tensor.matmul(
                            pq[:], WqT[dc][:, es], xT[dc][:, ss],
                            start=(dc == 0), stop=(dc == DC - 1),
                        )
                    nc.scalar.add(QT[ec][:, ss], pq[:], bqc[:, ec : ec + 1])
                    pk = psA.tile([P, F], F32, tag="psA", name="psA")
                    for dc in range(DC):
                        nc.tensor.matmul(
                            pk[:], WkT[dc][:, es], xT[dc][:, ss],
                            start=(dc == 0), stop=(dc == DC - 1),
                        )
                    nc.scalar.add(KT[ec][:, ss], pk[:], bkc[:, ec : ec + 1])

            for sc in range(SC):
                scs = slice(sc * P, (sc + 1) * P)
                pv = psA.tile([P, D], F32, tag="psA", name="psA")
                for dc in range(DC):
                    nc.tensor.matmul(
                        pv[:], xT[dc][:, scs], WvT[dc][:],
                        start=(dc == 0), stop=False,
                    )
                nc.tensor.matmul(pv[:], ones1[:], bvr[:], start=False, stop=True)
                nc.vector.tensor_copy(V[sc][:], pv[:])

            # ---- attention, blocked over q ------------------------------
            for qb in range(QB):
                qs = slice(qb * F, (qb + 1) * F)
                eblk = expp.tile([P, KC * F], BF16, tag="expblk", name="expblk")
                denp = recipp.tile([P, F], F32, tag="denp", name="denp")
                for kc in range(KC):
                    ks = slice(kc * P, (kc + 1) * P)
                    pss = psS.tile([P, F], F32, tag="psS", name="psS")
                    for ec in range(EC):
                        nc.tensor.matmul(
                            pss[:], KT[ec][:, ks], QT[ec][:, qs],
                            start=(ec == 0), stop=(ec == EC - 1),
                        )
                    nc.scalar.activation(
                        eblk[:, kc * F : (kc + 1) * F], pss[:],
                        mybir.ActivationFunctionType.Exp, scale=_SCALE,
                    )
                    # partial softmax denominator on DVE (keeps PE free):
                    # denp accumulates the per-partition chunk sums; the
                    # single ones-matmul below folds the partition axis.
                    if kc == 0:
                        nc.vector.tensor_copy(
                            denp[:], eblk[:, kc * F : (kc + 1) * F]
                        )
                    else:
                        nc.vector.tensor_add(
                            denp[:], denp[:], eblk[:, kc * F : (kc + 1) * F]
                        )

                denb = recipp.tile([P, F], BF16, tag="denb", name="denb")
                nc.vector.tensor_copy(denb[:], denp[:])
                pd = psS.tile([P, F], F32, tag="psS", name="psS")
                nc.tensor.matmul(pd[:], ones128[:], denb[:], start=True, stop=True)
                recip = recipp.tile([P, F], F32, tag="recip", name="recip")
                nc.vector.reciprocal_approx_fast(recip[:], pd[:])

                for ec in range(EC):
                    es = slice(ec * P, (ec + 1) * P)
                    pc = psC.tile([P, F], F32, tag="psC", name="psC")
                    for kc in range(KC):
                        nc.tensor.matmul(
                            pc[:], V[kc][:, es], eblk[:, kc * F : (kc + 1) * F],
                            start=(kc == 0), stop=(kc == KC - 1),
                        )
                    nc.vector.tensor_mul(ctxT[ec][:, qs], pc[:], recip[:])

                for sj in range(QB):
                    s0 = qb * F + sj * P
                    po = psA.tile([P, DO], F32, tag="psA", name="psA")
                    for ec in range(EC):
                        nc.tensor.matmul(
                            po[:], ctxT[ec][:, s0 : s0 + P], WoT[ec][:],
                            start=(ec == 0), stop=False,
                        )
                    nc.tensor.matmul(po[:], ones1[:], bor[:], start=False, stop=True)
                    ot = outp.tile([P, DO], F32, tag="out", name="outtile")
                    nc.scalar.copy(ot[:], po[:])
                    nc.sync.dma_start(out_e[s0 : s0 + P, :], ot[:])

    nc.compile()
    return nc


_NC = None


def _get_nc():
    global _NC
    if _NC is None:
        _NC = build()
    return _NC


def _make_in_maps(x, Wq, bq, Wk, bk, Wv, bv, Wo, bo):
    # Layout-only host prep: per-core shard = one batch element, transposed
    # weight/activation layouts (f32 throughout; casts happen on device).
    WqT = np.ascontiguousarray(np.asarray(Wq, np.float32).T)
    WkT = np.ascontiguousarray(np.asarray(Wk, np.float32).T)
    WvT = np.ascontiguousarray(np.asarray(Wv, np.float32).T)
    WoT = np.ascontiguousarray(np.asarray(Wo, np.float32).T)
    bq_col = np.ascontiguousarray(np.asarray(bq, np.float32).reshape(D, 1))
    bk_col = np.ascontiguousarray(np.asarray(bk, np.float32).reshape(D, 1))
    bv_row = np.ascontiguousarray(np.asarray(bv, np.float32).reshape(1, D))
    bo_row = np.ascontiguousarray(np.asarray(bo, np.float32).reshape(1, DO))
    in_maps = []
    for i in range(N_CORES):
        in_maps.append(
            {
                "xT": np.ascontiguousarray(np.asarray(x[i], np.float32).T),
                "WqT": WqT,
                "WkT": WkT,
                "WvT": WvT,
                "WoT": WoT,
                "bq_col": bq_col,
                "bk_col": bk_col,
                "bv_row": bv_row,
                "bo_row": bo_row,
            }
        )
    return in_maps


def run(inputs, trace=False):
    """Compile (cached) + run on cores 0-7. Returns (output, BassKernelResults)."""
    from concourse.bass_utils import run_bass_kernel_spmd

    nc = _get_nc()
    in_maps = _make_in_maps(**inputs)
    res = run_bass_kernel_spmd(
        nc, in_maps, core_ids=list(range(N_CORES)), trace=trace
    )
    out = np.stack([res.results[i]["out"] for i in range(N_CORES)], axis=0)
    return out.astype(np.float32), res


def kernel(**inputs) -> np.ndarray:
    out, _ = run(inputs, trace=False)
    return out


# revision 9
# speedup vs baseline: 1.2401x; 1.1600x over previous
"""Single-head attention (B=8, S=2048, D=512) on 8 TRN2 NeuronCores.

Sharding: data-parallel over batch — core i computes batch element i
entirely locally (no collectives). Host-side prep is layout only
(transpose/reshape of the f32 shards); all compute (casts, projections,
attention, softmax, output projection) runs on-device.

Math (per core, x = x[b] of shape [S, D]):
  Q^T[e,s] = sum_d WqT[d,e] xT[d,s] + bq[e]      (bf16 matmul, f32 psum)
  K^T, V analogous; V kept as [s,e].
  S^T[k,q] = sum_e K^T[e,k] Q^T[e,q]             (scores, transposed layout)
  E = exp(S^T / sqrt(D))                          (ScalarE, no max-sub: scores
                                                   are O(30) max -> exp fits f32)
  denom[q] = sum_k E[k,q] via all-ones matmul -> replicated over partitions
  ctx^T[e,q] = sum_k V[k,e] E[k,q]; normalized by 1/denom during psum->sbuf
  out[s,o] = sum_e ctx^T[e,s] WoT[e,o] + bo      -> DMA to DRAM
"""

import sys

if "/opt/trn_rl_repo" not in sys.path:
    sys.path.insert(0, "/opt/trn_rl_repo")

import math

import numpy as np

import concourse.bass as bass
import concourse.mybir as mybir
import concourse.tile as tile

from concourse import bacc
from concourse.tile import TileContext

N_CORES = 8
S = 2048
D = 512
DO = 512

P = 128          # partition tile
F = 512          # free-dim tile (psum bank = 512 f32)
DC = D // P      # 4 contraction chunks over d
EC = D // P      # 4 chunks over e
SC = S // P      # 16 chunks over s (=k)
QB = S // F      # 4 q blocks of 512
KC = S // P      # 16 k chunks

F32 = mybir.dt.float32
BF16 = mybir.dt.bfloat16

_SCALE = 1.0 / math.sqrt(D)


def build():
    nc = bacc.Bacc(None)

    xT_e = nc.dram_tensor("xT", [D, S], F32, kind="ExternalInput")
    WqT_e = nc.dram_tensor("WqT", [D, D], F32, kind="ExternalInput")
    WkT_e = nc.dram_tensor("WkT", [D, D], F32, kind="ExternalInput")
    WvT_e = nc.dram_tensor("WvT", [D, D], F32, kind="ExternalInput")
    WoT_e = nc.dram_tensor("WoT", [D, DO], F32, kind="ExternalInput")
    bq_e = nc.dram_tensor("bq_col", [D, 1], F32, kind="ExternalInput")
    bk_e = nc.dram_tensor("bk_col", [D, 1], F32, kind="ExternalInput")
    bv_e = nc.dram_tensor("bv_row", [1, D], F32, kind="ExternalInput")
    bo_e = nc.dram_tensor("bo_row", [1, DO], F32, kind="ExternalInput")
    out_e = nc.dram_tensor("out", [S, DO], F32, kind="ExternalOutput")

    with TileContext(nc) as tc:
        with (
            tc.tile_pool(name="io", bufs=2) as io,
            tc.tile_pool(name="persist", bufs=1) as ps,
            tc.tile_pool(name="expp", bufs=2) as expp,
            tc.tile_pool(name="recipp", bufs=2) as recipp,
            tc.tile_pool(name="outp", bufs=3) as outp,
            tc.tile_pool(name="psA", bufs=2, space="PSUM") as psA,
            tc.tile_pool(name="psS", bufs=3, space="PSUM") as psS,
            tc.tile_pool(name="psC", bufs=3, space="PSUM") as psC,
        ):
            # ---- load + cast inputs -------------------------------------
            xT = []
            for dc in range(DC):
                xl = io.tile([P, S], F32, tag="xload", name="xload")
                nc.sync.dma_start(xl[:], xT_e[dc * P : (dc + 1) * P, :])
                xb = ps.tile([P, S], BF16, tag=f"xT{dc}", name=f"xT{dc}")
                nc.vector.tensor_copy(xb[:], xl[:])
                xT.append(xb)

            def load_w(ext, name, ncols):
                tiles = []
                for dc in range(DC):
                    wl = io.tile([P, ncols], F32, tag="wload", name="wload")
                    nc.sync.dma_start(wl[:], ext[dc * P : (dc + 1) * P, :])
                    wb = ps.tile([P, ncols], BF16, tag=f"{name}{dc}", name=f"{name}{dc}")
                    nc.vector.tensor_copy(wb[:], wl[:])
                    tiles.append(wb)
                return tiles

            WqT = load_w(WqT_e, "WqT", D)
            WkT = load_w(WkT_e, "WkT", D)
            WvT = load_w(WvT_e, "WvT", D)
            WoT = load_w(WoT_e, "WoT", DO)

            bqc = ps.tile([P, DC], F32, tag="bqc", name="bqc")
            bkc = ps.tile([P, DC], F32, tag="bkc", name="bkc")
            for j in range(DC):
                nc.sync.dma_start(bqc[:, j : j + 1], bq_e[j * P : (j + 1) * P, :])
                nc.sync.dma_start(bkc[:, j : j + 1], bk_e[j * P : (j + 1) * P, :])

            bvl = io.tile([1, D], F32, tag="brow", name="brow")
            nc.sync.dma_start(bvl[:], bv_e[:, :])
            bvr = ps.tile([1, D], BF16, tag="bvr", name="bvr")
            nc.vector.tensor_copy(bvr[:], bvl[:])
            bol = io.tile([1, DO], F32, tag="brow", name="brow")
            nc.sync.dma_start(bol[:], bo_e[:, :])
            bor = ps.tile([1, DO], BF16, tag="bor", name="bor")
            nc.vector.tensor_copy(bor[:], bol[:])

            ones1 = ps.tile([1, P], BF16, tag="ones1", name="ones1")
            nc.any.memset(ones1[:], 1.0)
            ones128 = ps.tile([P, P], BF16, tag="ones128", name="ones128")
            nc.any.memset(ones128[:], 1.0)

            # ---- QKV projections ----------------------------------------
            QT = [ps.tile([P, S], BF16, tag=f"QT{ec}", name=f"QT{ec}") for ec in range(EC)]
            KT = [ps.tile([P, S], BF16, tag=f"KT{ec}", name=f"KT{ec}") for ec in range(EC)]
            V = [ps.tile([P, D], BF16, tag=f"V{sc}", name=f"V{sc}") for sc in range(SC)]
            ctxT = [ps.tile([P, S], BF16, tag=f"ctxT{ec}", name=f"ctxT{ec}") for ec in range(EC)]

            for ec in range(EC):
                es = slice(ec * P, (ec + 1) * P)
                for sb in range(QB):
                    ss = slice(sb * F, (sb + 1) * F)
                    pq = psA.tile([P, F], F32, tag="psA", name="psA")
                    for dc in range(DC):
                        nc.tensor.matmul(
                            pq[:], WqT[dc][:, es], xT[dc][:, ss],
                            start=(dc == 0), stop=(dc == DC - 1),
                        )
                    nc.scalar.add(QT[ec][:, ss], pq[:], bqc[:, ec : ec + 1])
                    pk = psA.tile([P, F], F32, tag="psA", name="psA")
                    for dc in range(DC):
                        nc.tensor.matmul(
                            pk[:], WkT[dc][:, es], xT[dc][:, ss],
                            start=(dc == 0), stop=(dc == DC - 1),
                        )
                    nc.scalar.add(KT[ec][:, ss], pk[:], bkc[:, ec : ec + 1])

            for sc in range(SC):
                scs = slice(sc * P, (sc + 1) * P)
                pv = psA.tile([P, D], F32, tag="psA", name="psA")
                for dc in range(DC):
                    nc.tensor.matmul(
                        pv[:], xT[dc][:, scs], WvT[dc][:],
                        start=(dc == 0), stop=False,
                    )
                nc.tensor.matmul(pv[:], ones1[:], bvr[:], start=False, stop=True)
                nc.vector.tensor_copy(V[sc][:], pv[:])

            # ---- attention, blocked over q ------------------------------
            for qb in range(QB):
                qs = slice(qb * F, (qb + 1) * F)
                eblk = expp.tile([P, KC * F], BF16, tag="expblk", name="expblk")
                denp = recipp.tile([P, F], F32, tag="denp", name="denp")
                for kc in range(KC):
                    ks = slice(kc * P, (kc + 1) * P)
                    pss = psS.tile([P, F], F32, tag="psS", name="psS")
                    for ec in range(EC):
                        nc.tensor.matmul(
                            pss[:], KT[ec][:, ks], QT[ec][:, qs],
                            start=(ec == 0), stop=(ec == EC - 1),
                        )
                    nc.scalar.activation(
                        eblk[:, kc * F : (kc + 1) * F], pss[:],
                        mybir.ActivationFunctionType.Exp, scale=_SCALE,
                    )
                    # partial softmax denominator on DVE (keeps PE free):
                    # denp accumulates the per-partition chunk sums; the
                    # single ones-matmul below folds the partition axis.
                    if kc == 0:
                        nc.vector.tensor_copy(
                            denp[:], eblk[:, kc * F : (kc + 1) * F]
                        )
                    else:
                        nc.vector.tensor_add(
                            denp[:], denp[:], eblk[:, kc * F : (kc + 1) * F]
                        )

                denb = recipp.tile([P, F], BF16, tag="denb", name="denb")
                nc.vector.tensor_copy(denb[:], denp[:])
                pd = psS.tile([P, F], F32, tag="psS", name="psS")
                nc.tensor.matmul(pd[:], ones128[:], denb[:], start=True, stop=True)
                recip = recipp.tile([P, F], F32, tag="recip", name="recip")
                nc.vector.reciprocal_approx_fast(recip[:], pd[:])

                for ec in range(EC):
                    es = slice(ec * P, (ec + 1) * P)
                    pc = psC.tile([P, F], F32, tag="psC", name="psC")
                    for kc in range(KC):
                        nc.tensor.matmul(
                            pc[:], V[kc][:, es], eblk[:, kc * F : (kc + 1) * F],
                            start=(kc == 0), stop=(kc == KC - 1),
                        )
                    nc.vector.tensor_mul(ctxT[ec][:, qs], pc[:], recip[:])

                for sj in range(QB):
                    s0 = qb * F + sj * P
                    po = psA.tile([P, DO], F32, tag="psA", name="psA")
                    for ec in range(EC):
                        nc.tensor.matmul(
                            po[:], ctxT[ec][:, s0 : s0 + P], WoT[ec][:],
                            start=(ec == 0), stop=False,
                        )
                    nc.tensor.matmul(po[:], ones1[:], bor[:], start=False, stop=True)
                    ot = outp.tile([P, DO], F32, tag="out", name="outtile")
                    nc.scalar.copy(ot[:], po[:])
                    nc.sync.dma_start(out_e[s0 : s0 + P, :], ot[:])

    nc.compile()
    return nc


_NC = None


def _get_nc():
    global _NC
    if _NC is None:
        _NC = build()
    return _NC


def _make_in_maps(x, Wq, bq, Wk, bk, Wv, bv, Wo, bo):
    # Layout-only host prep: per-core shard = one batch element, transposed
    # weight/activation layouts (f32 throughout; casts happen on device).
    WqT = np.ascontiguousarray(np.asarray(Wq, np.float32).T)
    WkT = np.ascontiguousarray(np.asarray(Wk, np.float32).T)
    WvT = np.ascontiguousarray(np.asarray(Wv, np.float32).T)
    WoT = np.ascontiguousarray(np.asarray(Wo, np.float32).T)
    bq_col = np.ascontiguousarray(np.asarray(bq, np.float32).reshape(D, 1))
    bk_col = np.ascontiguousarray(np.asarray(bk, np.float32).reshape(D, 1))
    bv_row = np.ascontiguousarray(np.asarray(bv, np.float32).reshape(1, D))
    bo_row = np.ascontiguousarray(np.asarray(bo, np.float32).reshape(1, DO))
    in_maps = []
    for i in range(N_CORES):
        in_maps.append(
            {
                "xT": np.ascontiguousarray(np.asarray(x[i], np.float32).T),
                "WqT": WqT,
                "WkT": WkT,
                "WvT": WvT,
                "WoT": WoT,
                "bq_col": bq_col,
                "bk_col": bk_col,
                "bv_row": bv_row,
                "bo_row": bo_row,
            }
        )
    return in_maps


def run(inputs, trace=False):
    """Compile (cached) + run on cores 0-7. Returns (output, BassKernelResults)."""
    from concourse.bass_utils import run_bass_kernel_spmd

    nc = _get_nc()
    in_maps = _make_in_maps(**inputs)
    res = run_bass_kernel_spmd(
        nc, in_maps, core_ids=list(range(N_CORES)), trace=trace
    )
    out = np.stack([res.results[i]["out"] for i in range(N_CORES)], axis=0)
    return out.astype(np.float32), res


def kernel(**inputs) -> np.ndarray:
    out, _ = run(inputs, trace=False)
    return out
